# revision 2
# baseline (speedup 1.0000x reference)
"""NRI-style GNN encoder (gnn_message_passing) on 8 Trainium2 NeuronCores.

Data-parallel over batch: core b computes batch element b end-to-end.

V2 restructure (vs V1 baseline's one-hot gather matmuls):
  - The graph is fully-connected-minus-diagonal, so edge e=(i,j) enumerates a
    128x128 pair grid. We compute the PADDED grid (P=16384 pairs incl. the
    diagonal) and drop diagonal rows on the host. Gathers vanish:
    edge-MLP-1 pre-activations are the outer sum u_i + v_j.
  - elu(y)+1 = min(exp(y), max(y+1,1)) (exact). For pass-1 stage A,
    exp(u+v) = exp(u)*exp(v) -- an outer product -- so NO dense exp is needed
    there; t and r come from per-block tensor_scalar ops with per-partition
    scalar columns (4x DVE mode / Pool).
  - Everything lives transposed [feature(128 part) x 2 halves x pairs]; z1
    stays SBUF-resident (64KB/partition): no DRAM spill, no rel matrices,
    no PE transposes.
  - Aggregation = per-block sum of z1 = TT-add tree + strided diagonal
    subtraction (feeds node stage 2 directly in transposed layout).
  - Pass-2 layer 1: skip term C2^T z1 via PE; v2/u2 outer-sum terms ride two
    more matmuls against tiny structural patterns (tiled eye / block
    indicator); psum then holds y2+1 whole.
  - ELU sites are computed in one of two exact forms, assigned per-site to
    balance engines:
      FORM-D: t=ACT Exp(psum,b); t'=TSP(t min 1) [DVE 4x];
              z=STT(psum + b+1) max t' [DVE]
      FORM-P: same t,t'; r0=ACT Relu(psum-1); z=Pool STT (r0+1) max t'
"""

import os
import sys

for _p in ("/opt/trn_rl_repo",):
    if _p not in sys.path:
        sys.path.insert(0, _p)

import numpy as np

import concourse.bass as bass
import concourse.tile as tile
from concourse import bacc, mybir
from concourse.bass_utils import run_bass_kernel_spmd

DT = mybir.dt
AF = mybir.ActivationFunctionType
ALU = mybir.AluOpType

B, N, T, D, H, NE = 8, 128, 49, 4, 256, 2
E = N * (N - 1)          # 16256 real edges
P = N * N                # 16384 padded pairs
F = T * D                # 196
C1 = 1024                # pass-1 chunk (pairs)
C2C = 512                # pass-2 chunk (pairs); 32 chunks, 4 blocks each


def _mk_layout(entries):
    out, c = {}, 0
    for name, w in entries:
        out[name] = (c, w)
        c += w
    return out, c

PK32, CC32 = _mk_layout([
    ("ey32", 128), ("wn1a", 256), ("wn1b", 256),
    ("wn1l2", 512), ("a1s", 512), ("b1s", 512),
    ("wn2l1", 512), ("wn2l2", 512), ("a2s", 512), ("b2s", 512),
    ("nbs", 1024), ("b3t", 256), ("bos", 8), ("scal", 16),
])
# scal columns (f32, per-partition rows 0:128 = per-feature of each half):
#  0,1: be1 (fh=0,1)   2,3: be1+1   4,5: be2   6,7: be2+1
#  8,9: be4   10,11: be4+1   12..15: spare
PK16, CC16 = _mk_layout([
    ("we1l2", 512), ("c2s", 512), ("we2l2", 512),
    ("ows", 4), ("owsn", 4), ("ws2", 4), ("eyet", 512),
])

_PROG_CACHE = {}
LAST_EXEC_NS = None


def _build_program():
    nc = bacc.Bacc(
        "TRN2",
        target_bir_lowering=False,
        debug=False,
        enable_asserts=True,
        num_devices=8,
    )

    f32, f16 = DT.float32, DT.float16

    def din(name, shape, dt=f32):
        return nc.dram_tensor(name, list(shape), dt, kind="ExternalInput").ap()

    x_in = din("x_nm", [N, F])
    pk32 = din("pk32", [128, CC32], f32)
    pk16 = din("pk16", [128, CC16], f16)
    ueye_d = din("ueye", [128, P], f16)      # col 128*b+j -> 1 at row b

    out_d = nc.dram_tensor("out", [P, NE], f32, kind="ExternalOutput").ap()

    with tile.TileContext(nc) as tc:
        with (
            tc.tile_pool(name="const", bufs=1) as cpool,
            tc.tile_pool(name="z1p", bufs=1) as z1pool,
            tc.tile_pool(name="wk", bufs=2) as wk,
        ):
            # ---------- constants ----------
            p32 = cpool.tile([128, CC32], f32, name="p32")
            nc.sync.dma_start(p32[:], pk32)
            p16 = cpool.tile([128, CC16], f16, name="p16")
            nc.sync.dma_start(p16[:], pk16)
            x_sb = cpool.tile([N, F], f32, name="x_sb")
            nc.sync.dma_start(x_sb[:], x_in)

            def c32(name, hview=False):
                c0, w = PK32[name]
                ap = p32[:, c0:c0 + w]
                if hview:
                    ap = ap.rearrange("p (h o) -> p h o", h=2)
                return ap

            def c16(name, hview=False):
                c0, w = PK16[name]
                ap = p16[:, c0:c0 + w]
                if hview:
                    ap = ap.rearrange("p (h o) -> p h o", h=2)
                return ap

            ey32 = c32("ey32")
            wn1a = c32("wn1a")
            wn1b = c32("wn1b")[0:68, :]
            wn1l2 = c32("wn1l2", hview=True)
            a1s = c32("a1s", hview=True)
            b1s = c32("b1s", hview=True)
            wn2l1 = c32("wn2l1", hview=True)
            wn2l2 = c32("wn2l2", hview=True)
            a2s = c32("a2s", hview=True)
            b2s = c32("b2s", hview=True)
            nbs = c32("nbs").rearrange("p (h o) -> p h o", h=4)
            b3t = c32("b3t")
            bos = c32("bos")
            scal = c32("scal")
            we1l2 = c16("we1l2", hview=True)
            c2s = c16("c2s", hview=True)
            we2l2 = c16("we2l2", hview=True)
            ows = c16("ows", hview=True)
            owsn = c16("owsn", hview=True)
            ws2 = c16("ws2", hview=True)
            eyet = c16("eyet")

            def sc(i):
                return scal[:, i:i + 1]

            # z1 slab: transposed [f(128), 2 halves, P] fp16
            z1 = z1pool.tile([128, 2, P], f16, name="z1")
            aggt = cpool.tile([128, 2, 128], f32, name="aggt")

            # PE p-state trickle: tiny dependency-free matmuls keep the
            # tensor engine's clock ramped through pipeline gaps.
            def trickle(pool, n=1):
                tt = pool.tile([2, 4], f32, name="trk", tag="trk")
                for _ in range(n):
                    nc.tensor.matmul(tt[:, 0:4], p16[:, 0:2], p16[:, 0:4],
                                     start=True, stop=True,
                                     skip_group_check=True)

            # ---------- node-stage helpers ----------
            def node_mm(pool, lhsT, rhs):
                ps = pool.tile([128, 512], f32, name="ps_n", tag="npre")
                for fh in range(2):
                    nc.tensor.matmul(ps[:, :256], lhsT[:, fh], rhs[:, fh],
                                     start=(fh == 0), stop=(fh == 1))
                return ps

            def elu_node(ps, bcol_h, name):
                """z = elu(ps + nbs_k)+1 on [128,256] (all f32, tiny)."""
                y = wk.tile([128, 256], f32, name=name + "_y", tag="n_y")
                nc.vector.tensor_tensor(y[:], ps[:, :256], bcol_h, ALU.add)
                t = wk.tile([128, 256], f32, name=name + "_t", tag="n_t")
                nc.scalar.activation(t[:], y[:], AF.Exp)
                r = wk.tile([128, 256], f32, name=name + "_r", tag="n_r")
                nc.vector.tensor_scalar(r[:], y[:], 1.0, 1.0, ALU.add, ALU.max)
                z = cpool.tile([128, 256], f32, name=name)
                nc.vector.tensor_tensor(z[:], t[:], r[:], ALU.min)
                return z

            def tpose256(pool, src_ap, out_name, dt_out=f32):
                """[128n, 256f] -> [128f(part), 2 fh, 128n]."""
                ps = pool.tile([128, 512], f32, name="ps_tp", tag="ntp")
                for fh in range(2):
                    nc.tensor.transpose(ps[:, fh * 128:(fh + 1) * 128],
                                        src_ap[:, fh * 128:(fh + 1) * 128],
                                        ey32)
                t = cpool.tile([128, 2, 128], dt_out, name=out_name)
                nc.vector.tensor_copy(t[:].rearrange("p a b -> p (a b)"),
                                      ps[:, :256])
                return t

            # ---------- node stage 1 ----------
            with (
                tc.tile_pool(name="n1pre", bufs=2, space="PSUM") as npre,
                tc.tile_pool(name="n1tp", bufs=2, space="PSUM") as ntp,
            ):
                ps_x = ntp.tile([128, 512], f32, name="ps_x", tag="ntp")
                nc.tensor.transpose(ps_x[:, 0:128], x_sb[:, 0:128], ey32)
                nc.tensor.transpose(ps_x[0:68, 128:256], x_sb[:, 128:196],
                                    ey32)
                xt0 = cpool.tile([128, 128], f32, name="xt0")
                nc.vector.tensor_copy(xt0[:], ps_x[:, 0:128])
                xt1 = cpool.tile([68, 128], f32, name="xt1")
                nc.vector.tensor_copy(xt1[:], ps_x[0:68, 128:256])

                ps1 = npre.tile([128, 512], f32, name="ps_n", tag="npre")
                nc.tensor.matmul(ps1[:, :256], xt0[:], wn1a[:],
                                 start=True, stop=False)
                nc.tensor.matmul(ps1[:, :256], xt1[:], wn1b[:],
                                 start=False, stop=True)
                zh1a = elu_node(ps1, nbs[:, 0, :], "zh1a")
                zh1aT = tpose256(ntp, zh1a[:], "zh1aT")

                ps2 = node_mm(npre, zh1aT, wn1l2)
                zh1 = elu_node(ps2, nbs[:, 1, :], "zh1")
                zh1T = tpose256(ntp, zh1[:], "zh1T")

                ps_u1 = node_mm(npre, zh1T, a1s)
                u1sb = cpool.tile([128, 256], f32, name="u1sb")
                nc.scalar.copy(u1sb[:], ps_u1[:, :256])
                u1T = tpose256(ntp, u1sb[:], "u1T")
                ps_v1 = node_mm(npre, zh1T, b1s)
                v1sb = cpool.tile([128, 256], f32, name="v1sb")
                nc.scalar.copy(v1sb[:], ps_v1[:, :256])
                v1T = tpose256(ntp, v1sb[:], "v1T")

                # derived per-node tiles for pass 1 (tiny)
                e_u1 = cpool.tile([128, 2, 128], f32, name="e_u1")
                e_v1 = cpool.tile([128, 2, 128], f16, name="e_v1")
                u1p = cpool.tile([128, 2, 128], f32, name="u1p")
                v1t6 = cpool.tile([128, 2, 128], f16, name="v1t6")
                for fh in range(2):
                    nc.scalar.activation(e_u1[:, fh, :], u1T[:, fh, :],
                                         AF.Exp, bias=sc(0 + fh))
                    nc.scalar.activation(e_v1[:, fh, :], v1T[:, fh, :],
                                         AF.Exp)
                    nc.vector.tensor_scalar(u1p[:, fh, :], u1T[:, fh, :],
                                            sc(2 + fh), None, ALU.add)
                    nc.vector.tensor_copy(v1t6[:, fh, :], v1T[:, fh, :])

            # ---------- pass 1 over the padded pair grid ----------
            NB1 = C1 // 128  # 8 blocks per chunk
            with tc.tile_pool(name="p1ps", bufs=2, space="PSUM") as p1ps:
                def p1_stageA(c):
                    """t1 = e_u (x) e_v [DVE 4x]; r1 = max(u'+v+1,1) [Pool];
                    z1a = min(t1, r1) [DVE 2x]."""
                    t1 = wk.tile([128, 2, NB1, 128], f16, name="t1", tag="t1",
                                 bufs=2)
                    r1 = wk.tile([128, 2, NB1, 128], f16, name="r1", tag="r1",
                                 bufs=2)
                    for fh in range(2):
                        for b in range(NB1):
                            col = c * NB1 + b
                            nc.gpsimd.tensor_scalar(
                                r1[:, fh, b, :], v1t6[:, fh, :],
                                u1p[:, fh, col:col + 1], 1.0,
                                ALU.add, ALU.max)
                            nc.vector.tensor_scalar(
                                t1[:, fh, b, :], e_v1[:, fh, :],
                                e_u1[:, fh, col:col + 1], None, ALU.mult)
                    z1a = wk.tile([128, 2, C1], f16, name="z1a", tag="z1a",
                                  bufs=2)
                    nc.vector.tensor_tensor(
                        z1a[:].rearrange("p a b -> p (a b)"),
                        t1[:].rearrange("p a b j -> p (a b j)"),
                        r1[:].rearrange("p a b j -> p (a b j)"), ALU.min)
                    return z1a

                def p1_stageB(c, z1a):
                    ps = p1ps.tile([128, 2, C1], f32, name="ps1b", tag="p1")
                    for oh in range(2):
                        for q in range(0, C1, 512):
                            for fh in range(2):
                                nc.tensor.matmul(
                                    ps[:, oh, q:q + 512],
                                    we1l2[:, fh, oh * 128:(oh + 1) * 128],
                                    z1a[:, fh, q:q + 512],
                                    start=(fh == 0), stop=(fh == 1))
                    t1b = wk.tile([128, 2, C1], f16, name="t1b", tag="t1b",
                                  bufs=2)
                    r0b = wk.tile([128, 2, C1], f16, name="r0b", tag="r0b",
                                  bufs=2)
                    for oh in range(2):
                        nc.scalar.activation(t1b[:, oh, :], ps[:, oh, :],
                                             AF.Exp, bias=sc(4 + oh))
                        nc.scalar.activation(r0b[:, oh, :], ps[:, oh, :],
                                             AF.Relu, bias=sc(4 + oh))
                    rp = wk.tile([128, 2, C1], f16, name="rp", tag="rp",
                                 bufs=2)
                    nc.vector.tensor_scalar(
                        rp[:].rearrange("p a b -> p (a b)"),
                        r0b[:].rearrange("p a b -> p (a b)"),
                        1.0, None, ALU.add)
                    zs = z1[:, :, c * C1:(c + 1) * C1]
                    nc.vector.tensor_tensor(zs, t1b[:], rp[:], ALU.min)
                    # agg tree: per-block sums of z1 -> aggt columns
                    zb = zs.rearrange("p h (b j) -> p h b j", j=128)
                    s64 = wk.tile([128, 2, NB1, 64], f16, name="s64",
                                  tag="s64", bufs=2)
                    nc.vector.tensor_tensor(
                        s64[:], zb[:, :, :, 0:64], zb[:, :, :, 64:128],
                        ALU.add)
                    s32 = wk.tile([128, 2, NB1, 32], f16, name="s32",
                                  tag="s32", bufs=2)
                    nc.vector.tensor_tensor(
                        s32[:], s64[:, :, :, 0:32], s64[:, :, :, 32:64],
                        ALU.add)
                    s16 = wk.tile([128, 2, NB1, 16], f16, name="s16",
                                  tag="s16", bufs=2)
                    nc.vector.tensor_tensor(
                        s16[:], s32[:, :, :, 0:16], s32[:, :, :, 16:32],
                        ALU.add)
                    nc.vector.reduce_sum(
                        aggt[:, :, c * NB1:(c + 1) * NB1], s16[:],
                        axis=mybir.AxisListType.X)

                hist = []
                for c in range(P // C1):
                    hist.append(p1_stageA(c))
                    if c >= 1:
                        p1_stageB(c - 1, hist[c - 1])
                p1_stageB(P // C1 - 1, hist[-1])

            # diagonal correction: agg counted z1[i,i]; subtract it
            diag16 = cpool.tile([128, 2, 128], f16, name="diag16")
            zflat = z1[:].rearrange("p h e -> p (h e)")
            for h in range(2):
                nc.gpsimd.tensor_copy(
                    diag16[:, h, :],
                    zflat[:, h * P:h * P + P:129])
            aggc = cpool.tile([128, 2, 128], f32, name="aggc")
            nc.vector.tensor_tensor(
                aggc[:].rearrange("p a b -> p (a b)"),
                aggt[:].rearrange("p a b -> p (a b)"),
                diag16[:].rearrange("p a b -> p (a b)"), ALU.subtract)

            # ---------- node stage 2 ----------
            with (
                tc.tile_pool(name="n2pre", bufs=2, space="PSUM") as npre2,
                tc.tile_pool(name="n2tp", bufs=2, space="PSUM") as ntp2,
            ):
                ps3 = node_mm(npre2, aggc, wn2l1)
                zh2a = elu_node(ps3, nbs[:, 2, :], "zh2a")
                zh2aT = tpose256(ntp2, zh2a[:], "zh2aT")
                ps4 = node_mm(npre2, zh2aT, wn2l2)
                zh2 = elu_node(ps4, nbs[:, 3, :], "zh2")
                zh2T = tpose256(ntp2, zh2[:], "zh2T")

                ps_u2 = node_mm(npre2, zh2T, a2s)
                u2y = wk.tile([128, 256], f32, name="u2y", tag="n_y")
                nc.vector.tensor_tensor(u2y[:], ps_u2[:, :256], b3t, ALU.add)
                u2p6 = cpool.tile([128, 256], f16, name="u2p6")
                nc.scalar.copy(u2p6[:], u2y[:])
                ps_v2 = node_mm(npre2, zh2T, b2s)
                v2n6 = cpool.tile([128, 256], f16, name="v2n6")
                nc.scalar.copy(v2n6[:], ps_v2[:, :256])

            # ---------- pass 2 ----------
            NB2 = C2C // 128  # 4 blocks per chunk
            with (
                tc.tile_pool(name="p2a", bufs=2, space="PSUM") as p2a,
                tc.tile_pool(name="p2b", bufs=1, space="PSUM") as p2b,
                tc.tile_pool(name="p2o", bufs=2, space="PSUM") as p2o,
            ):
                def p2_A1_mm(c):
                    lo = c * C2C
                    psA = p2a.tile([128, 2, C2C], f32, name="psA", tag="p2a")
                    uey = wk.tile([128, C2C], f16, name="uey", tag="uey",
                                  bufs=4)
                    nc.sync.dma_start(uey[:], ueye_d[:, lo:lo + C2C])
                    for fh in range(2):
                        fsl = slice(fh * 128, (fh + 1) * 128)
                        for hh in range(2):
                            nc.tensor.matmul(
                                psA[:, fh, :], c2s[:, hh, fsl],
                                z1[:, hh, lo:lo + C2C],
                                start=(hh == 0), stop=False)
                        nc.tensor.matmul(
                            psA[:, fh, :], v2n6[:, fsl], eyet[:],
                            start=False, stop=False)
                        nc.tensor.matmul(
                            psA[:, fh, :], u2p6[:, fsl], uey[:],
                            start=False, stop=True)
                    return psA

                def p2_A2_elu(c, psA):
                    # psA = y2 + 1
                    psAf = psA[:].rearrange("p a b -> p (a b)")
                    t2 = wk.tile([128, 2, C2C], f16, name="t2", tag="t2",
                                 bufs=2)
                    t2f = t2[:].rearrange("p a b -> p (a b)")
                    nc.scalar.activation(t2f, psAf, AF.Exp, bias=sc(12))
                    t2p = wk.tile([128, 2, C2C], f16, name="t2p", tag="t2p",
                                  bufs=2)
                    t2pf = t2p[:].rearrange("p a b -> p (a b)")
                    nc.vector.tensor_scalar(t2pf, t2f, 1.0, None, ALU.min)
                    # z2a = max(min(exp(y2),1), y2+1)
                    z2a = wk.tile([128, 2, C2C], f16, name="z2a", tag="z2a",
                                  bufs=3)
                    nc.vector.tensor_tensor(
                        z2a[:].rearrange("p a b -> p (a b)"), psAf, t2pf,
                        ALU.max)
                    return z2a

                def p2_B1_mm(c, z2a):
                    psB = p2b.tile([128, 2, C2C], f32, name="psB", tag="p2b")
                    for oh in range(2):
                        for fh in range(2):
                            nc.tensor.matmul(
                                psB[:, oh, :],
                                we2l2[:, fh, oh * 128:(oh + 1) * 128],
                                z2a[:, fh, :],
                                start=(fh == 0), stop=(fh == 1))
                    return psB

                def p2_B2_elu(c, psB):
                    # p2B sum-form: z2 = exp(m) + relu(y2b') with
                    # m = min(y2b + be4, 0); relu(y2b') = (y2b + be4) - m.
                    # Linear term folds through the output matmul:
                    # y2b^T ow = z2a^T (e2w2@ow); be4^T ow -> bos.
                    q2 = wk.tile([128, 2, C2C], f16, name="q2", tag="q2",
                                 bufs=3)
                    neg = (c % 4 != 0)
                    if not neg:
                        for oh in range(2):
                            nc.scalar.activation(q2[:, oh, :], psB[:, oh, :],
                                                 AF.Relu, bias=sc(13 + oh),
                                                 scale=-1.0)
                    else:
                        # q2 holds m = min(y2b + be4, 0)
                        for oh in range(2):
                            nc.vector.tensor_scalar(
                                q2[:, oh, :], psB[:, oh, :], sc(8 + oh), 0.0,
                                ALU.add, ALU.min)
                    t2b = wk.tile([128, 2, C2C], f16, name="t2b", tag="t2b",
                                  bufs=3)
                    nc.scalar.activation(
                        t2b[:].rearrange("p a b -> p (a b)"),
                        q2[:].rearrange("p a b -> p (a b)"), AF.Exp,
                        scale=(1.0 if neg else -1.0))
                    return (q2, t2b, neg)

                def p2_B3_out(c, z2a, qt):
                    q2, t2b, neg = qt
                    lo = c * C2C
                    op = p2o.tile([128, NB2 * NE], f32, name="op",
                                  tag="p2o")
                    for j in range(NB2):
                        jsl = slice(j * 128, (j + 1) * 128)
                        for hh in range(2):
                            nc.tensor.matmul(
                                op[:, 2 * j:2 * j + 2], t2b[:, hh, jsl],
                                ows[:, hh, :],
                                start=(hh == 0), stop=False)
                            nc.tensor.matmul(
                                op[:, 2 * j:2 * j + 2], q2[:, hh, jsl],
                                (owsn if neg else ows)[:, hh, :],
                                start=False, stop=False)
                            nc.tensor.matmul(
                                op[:, 2 * j:2 * j + 2], z2a[:, hh, jsl],
                                ws2[:, hh, :],
                                start=False, stop=(hh == 1))
                    osb = wk.tile([128, NB2 * NE], f32, name="osb",
                                  tag="osb", bufs=2)
                    nc.vector.tensor_tensor(osb[:], op[:], bos, ALU.add)
                    nc.sync.dma_start(
                        out_d[lo:lo + C2C, :].rearrange("(j p) c -> p j c",
                                                        p=128),
                        osb[:].rearrange("p (j c) -> p j c", c=NE))

                hist2 = []
                for c in range(P // C2C):
                    psA = p2_A1_mm(c)
                    hist2.append(p2_A2_elu(c, psA))
                    if c >= 1:
                        zz = hist2[c - 1]
                        p2_B3_out(c - 1, zz, p2_B2_elu(c - 1,
                                                      p2_B1_mm(c - 1, zz)))
                zz = hist2[-1]
                p2_B3_out(P // C2C - 1, zz,
                          p2_B2_elu(P // C2C - 1,
                                    p2_B1_mm(P // C2C - 1, zz)))

    nc.compile()
    return nc


def _prep_inputs(inputs):
    """Host-side constant preprocessing -> shared in_map (all cores)."""
    f = lambda a: np.ascontiguousarray(np.asarray(a, dtype=np.float32))
    cs = lambda w: w.sum(axis=0)

    n1w1, n1b1 = f(inputs["n1w1"]), f(inputs["n1b1"])
    n1w2, n1b2 = f(inputs["n1w2"]), f(inputs["n1b2"])
    e1w1, e1b1 = f(inputs["e1w1"]), f(inputs["e1b1"])
    e1w2, e1b2 = f(inputs["e1w2"]), f(inputs["e1b2"])
    n2w1, n2b1 = f(inputs["n2w1"]), f(inputs["n2b1"])
    n2w2, n2b2 = f(inputs["n2w2"]), f(inputs["n2b2"])
    e2w1, e2b1 = f(inputs["e2w1"]), f(inputs["e2b1"])
    e2w2, e2b2 = f(inputs["e2w2"]), f(inputs["e2b2"])
    ow, ob = f(inputs["ow"]), f(inputs["ob"])

    A1, B1 = e1w1[:256], e1w1[256:]
    A2, B2, C2m = e2w1[:256], e2w1[256:512], e2w1[512:]

    e1w2_h = e1w2.astype(np.float16)
    C2_h = C2m.astype(np.float16)
    e2w2_h = e2w2.astype(np.float16)
    ow_h = ow.astype(np.float16)

    # activations are stored as z = elu+1; fold the -1 into consumer biases
    be1 = e1b1 - cs(A1) - cs(B1)
    be2 = e1b2 - cs(e1w2_h.astype(np.float32))
    be3 = e2b1 - cs(A2) - cs(B2) - cs(C2_h.astype(np.float32))
    be4 = e2b2 - cs(e2w2_h.astype(np.float32))
    ob_adj = ob - cs(ow_h.astype(np.float32))

    indeg = 127.0
    nbias = np.zeros((128, 4, 256), np.float32)
    nbias[:, 0, :] = n1b1[None, :]
    nbias[:, 1, :] = (n1b2 - cs(n1w2))[None, :]
    nbias[:, 2, :] = (n2b1 - indeg * cs(n2w1))[None, :]
    nbias[:, 3, :] = (n2b2 - cs(n2w2))[None, :]

    def sqh(w):  # [256, x] -> [128, 2*x] partition-major halves
        return np.ascontiguousarray(
            w.reshape(2, 128, -1).transpose(1, 0, 2).reshape(128, -1))

    pk32 = np.zeros((128, CC32), np.float32)

    def put32(name, arr):
        c0, w = PK32[name]
        pk32[:arr.shape[0], c0:c0 + w] = arr

    put32("ey32", np.eye(128, dtype=np.float32))
    put32("wn1a", n1w1[:128])
    put32("wn1b", n1w1[128:])
    put32("wn1l2", sqh(n1w2))
    put32("a1s", sqh(A1)); put32("b1s", sqh(B1))
    put32("wn2l1", sqh(n2w1)); put32("wn2l2", sqh(n2w2))
    put32("a2s", sqh(A2)); put32("b2s", sqh(B2))
    put32("nbs", nbias.reshape(128, -1))
    put32("b3t", np.tile((e2b1 - cs(A2) - cs(B2) - cs(C2_h.astype(np.float32))
                          + 1.0)[None, :], (128, 1)))
    bos_v = ob_adj + be4 @ ow_h.astype(np.float32)
    put32("bos", np.tile(bos_v[None, :], (128, 4)).astype(np.float32))
    scalv = np.zeros((128, 16), np.float32)
    for fh in range(2):
        sl = slice(fh * 128, (fh + 1) * 128)
        scalv[:, 0 + fh] = be1[sl]
        scalv[:, 2 + fh] = be1[sl] + 1.0
        scalv[:, 4 + fh] = be2[sl]
        scalv[:, 6 + fh] = be2[sl] + 1.0
        scalv[:, 8 + fh] = be4[sl]
        scalv[:, 10 + fh] = be4[sl] + 1.0
    scalv[:, 12] = -1.0
    for fh in range(2):
        scalv[:, 13 + fh] = -be4[fh * 128:(fh + 1) * 128]
    put32("scal", scalv)

    pk16 = np.zeros((128, CC16), np.float16)

    def put16(name, arr):
        c0, w = PK16[name]
        pk16[:arr.shape[0], c0:c0 + w] = arr

    put16("we1l2", sqh(e1w2_h.astype(np.float32)).astype(np.float16))
    put16("c2s", sqh(C2_h.astype(np.float32)).astype(np.float16))
    put16("we2l2", sqh(e2w2_h.astype(np.float32)).astype(np.float16))
    put16("ows", sqh(ow_h.astype(np.float32)).astype(np.float16))
    put16("owsn", sqh(-ow_h.astype(np.float32)).astype(np.float16))
    ws2f = e2w2_h.astype(np.float32) @ ow_h.astype(np.float32)
    put16("ws2", sqh(ws2f).astype(np.float16))
    put16("eyet", np.tile(np.eye(128, dtype=np.float16), (1, 4)))

    ueye = np.repeat(np.eye(128, dtype=np.float16), 128, axis=1)

    shared = dict(pk32=pk32, pk16=pk16,
                  ueye=np.ascontiguousarray(ueye))
    return shared


def kernel(**inputs):
    global LAST_EXEC_NS
    if "prog" not in _PROG_CACHE:
        _PROG_CACHE["prog"] = _build_program()
    nc = _PROG_CACHE["prog"]

    shared = _prep_inputs(inputs)
    x = np.asarray(inputs["x"], dtype=np.float32)
    in_maps = []
    for b in range(B):
        m = dict(shared)
        m["x_nm"] = np.ascontiguousarray(x[b].reshape(N, F))
        in_maps.append(m)

    trace = os.environ.get("KERNEL_TRACE", "0") == "1"
    try:
        res = run_bass_kernel_spmd(nc, in_maps, core_ids=list(range(8)),
                                   trace=trace)
    except ModuleNotFoundError:
        res = run_bass_kernel_spmd(nc, in_maps, core_ids=list(range(8)),
                                   trace=False)
    if trace and res.exec_time_ns is not None:
        LAST_EXEC_NS = res.exec_time_ns
        print(f"HW exec time: {res.exec_time_ns} ns "
              f"(mean {res.mean_exec_time_ns} ns, "
              f"slowest core {res.max_exec_time_core_id})")

    keep = ~np.eye(N, dtype=bool)
    outs = []
    for b in range(B):
        padded = res.results[b]["out"].reshape(N, N, NE)
        outs.append(padded[keep])
    return np.stack(outs, axis=0).astype(np.float32)


# revision 3
# speedup vs baseline: 1.0167x; 1.0167x over previous
"""NRI-style GNN encoder (gnn_message_passing) on 8 Trainium2 NeuronCores.

Data-parallel over batch: core b computes batch element b end-to-end.

V2 restructure (vs V1 baseline's one-hot gather matmuls):
  - The graph is fully-connected-minus-diagonal, so edge e=(i,j) enumerates a
    128x128 pair grid. We compute the PADDED grid (P=16384 pairs incl. the
    diagonal) and drop diagonal rows on the host. Gathers vanish:
    edge-MLP-1 pre-activations are the outer sum u_i + v_j.
  - elu(y)+1 = min(exp(y), max(y+1,1)) (exact). For pass-1 stage A,
    exp(u+v) = exp(u)*exp(v) -- an outer product -- so NO dense exp is needed
    there; t and r come from per-block tensor_scalar ops with per-partition
    scalar columns (4x DVE mode / Pool).
  - Everything lives transposed [feature(128 part) x 2 halves x pairs]; z1
    stays SBUF-resident (64KB/partition): no DRAM spill, no rel matrices,
    no PE transposes.
  - Aggregation = per-block sum of z1 = TT-add tree + strided diagonal
    subtraction (feeds node stage 2 directly in transposed layout).
  - Pass-2 layer 1: skip term C2^T z1 via PE; v2/u2 outer-sum terms ride two
    more matmuls against tiny structural patterns (tiled eye / block
    indicator); psum then holds y2+1 whole.
  - ELU sites are computed in one of two exact forms, assigned per-site to
    balance engines:
      FORM-D: t=ACT Exp(psum,b); t'=TSP(t min 1) [DVE 4x];
              z=STT(psum + b+1) max t' [DVE]
      FORM-P: same t,t'; r0=ACT Relu(psum-1); z=Pool STT (r0+1) max t'
"""

import os
import sys

for _p in ("/opt/trn_rl_repo",):
    if _p not in sys.path:
        sys.path.insert(0, _p)

import numpy as np

import concourse.bass as bass
import concourse.tile as tile
from concourse import bacc, mybir
from concourse.bass_utils import run_bass_kernel_spmd

DT = mybir.dt
AF = mybir.ActivationFunctionType
ALU = mybir.AluOpType

B, N, T, D, H, NE = 8, 128, 49, 4, 256, 2
E = N * (N - 1)          # 16256 real edges
P = N * N                # 16384 padded pairs
F = T * D                # 196
C1 = 1024                # pass-1 chunk (pairs)
C2C = 512                # pass-2 chunk (pairs); 32 chunks, 4 blocks each


def _mk_layout(entries):
    out, c = {}, 0
    for name, w in entries:
        out[name] = (c, w)
        c += w
    return out, c

PK32, CC32 = _mk_layout([
    ("ey32", 128), ("wn1a", 256), ("wn1b", 256),
    ("wn1l2", 512), ("a1s", 512), ("b1s", 512),
    ("wn2l1", 512), ("wn2l2", 512), ("a2s", 512), ("b2s", 512),
    ("nbs", 1024), ("b3t", 256), ("bos", 8), ("scal", 16),
])
# scal columns (f32, per-partition rows 0:128 = per-feature of each half):
#  0,1: be1 (fh=0,1)   2,3: be1+1   4,5: be2   6,7: be2+1
#  8,9: be4   10,11: be4+1   12..15: spare
PK16, CC16 = _mk_layout([
    ("we1l2", 512), ("c2s", 512), ("we2l2", 512),
    ("ows", 4), ("owsn", 4), ("ws2", 4), ("eyet", 512),
])

_PROG_CACHE = {}
LAST_EXEC_NS = None


def _build_program():
    nc = bacc.Bacc(
        "TRN2",
        target_bir_lowering=False,
        debug=False,
        enable_asserts=True,
        num_devices=8,
    )

    f32, f16 = DT.float32, DT.float16

    def din(name, shape, dt=f32):
        return nc.dram_tensor(name, list(shape), dt, kind="ExternalInput").ap()

    x_in = din("x_nm", [N, F])
    pk32 = din("pk32", [128, CC32], f32)
    pk16 = din("pk16", [128, CC16], f16)
    ueye_d = din("ueye", [128, P], f16)      # col 128*b+j -> 1 at row b

    out_d = nc.dram_tensor("out", [P, NE], f32, kind="ExternalOutput").ap()

    with tile.TileContext(nc) as tc:
        with (
            tc.tile_pool(name="const", bufs=1) as cpool,
            tc.tile_pool(name="z1p", bufs=1) as z1pool,
            tc.tile_pool(name="wk", bufs=2) as wk,
        ):
            # ---------- constants ----------
            p32 = cpool.tile([128, CC32], f32, name="p32")
            nc.sync.dma_start(p32[:], pk32)
            p16 = cpool.tile([128, CC16], f16, name="p16")
            nc.sync.dma_start(p16[:], pk16)
            x_sb = cpool.tile([N, F], f32, name="x_sb")
            nc.sync.dma_start(x_sb[:], x_in)

            def c32(name, hview=False):
                c0, w = PK32[name]
                ap = p32[:, c0:c0 + w]
                if hview:
                    ap = ap.rearrange("p (h o) -> p h o", h=2)
                return ap

            def c16(name, hview=False):
                c0, w = PK16[name]
                ap = p16[:, c0:c0 + w]
                if hview:
                    ap = ap.rearrange("p (h o) -> p h o", h=2)
                return ap

            ey32 = c32("ey32")
            wn1a = c32("wn1a")
            wn1b = c32("wn1b")[0:68, :]
            wn1l2 = c32("wn1l2", hview=True)
            a1s = c32("a1s", hview=True)
            b1s = c32("b1s", hview=True)
            wn2l1 = c32("wn2l1", hview=True)
            wn2l2 = c32("wn2l2", hview=True)
            a2s = c32("a2s", hview=True)
            b2s = c32("b2s", hview=True)
            nbs = c32("nbs").rearrange("p (h o) -> p h o", h=4)
            b3t = c32("b3t")
            bos = c32("bos")
            scal = c32("scal")
            we1l2 = c16("we1l2", hview=True)
            c2s = c16("c2s", hview=True)
            we2l2 = c16("we2l2", hview=True)
            ows = c16("ows", hview=True)
            owsn = c16("owsn", hview=True)
            ws2 = c16("ws2", hview=True)
            eyet = c16("eyet")

            def sc(i):
                return scal[:, i:i + 1]

            # z1 slab: transposed [f(128), 2 halves, P] fp16
            z1 = z1pool.tile([128, 2, P], f16, name="z1")
            aggt = cpool.tile([128, 2, 128], f32, name="aggt")

            # PE p-state trickle: tiny dependency-free matmuls keep the
            # tensor engine's clock ramped through pipeline gaps.
            def trickle(pool, n=1):
                tt = pool.tile([2, 4], f32, name="trk", tag="trk")
                for _ in range(n):
                    nc.tensor.matmul(tt[:, 0:4], p16[:, 0:2], p16[:, 0:4],
                                     start=True, stop=True,
                                     skip_group_check=True)

            # ---------- node-stage helpers ----------
            def node_mm(pool, lhsT, rhs):
                ps = pool.tile([128, 512], f32, name="ps_n", tag="npre")
                for fh in range(2):
                    nc.tensor.matmul(ps[:, :256], lhsT[:, fh], rhs[:, fh],
                                     start=(fh == 0), stop=(fh == 1))
                return ps

            def elu_node(ps, bcol_h, name):
                """z = elu(ps + nbs_k)+1 on [128,256] (all f32, tiny)."""
                y = wk.tile([128, 256], f32, name=name + "_y", tag="n_y")
                nc.vector.tensor_tensor(y[:], ps[:, :256], bcol_h, ALU.add)
                t = wk.tile([128, 256], f32, name=name + "_t", tag="n_t")
                nc.scalar.activation(t[:], y[:], AF.Exp)
                r = wk.tile([128, 256], f32, name=name + "_r", tag="n_r")
                nc.vector.tensor_scalar(r[:], y[:], 1.0, 1.0, ALU.add, ALU.max)
                z = cpool.tile([128, 256], f32, name=name)
                nc.vector.tensor_tensor(z[:], t[:], r[:], ALU.min)
                return z

            def tpose256(pool, src_ap, out_name, dt_out=f32):
                """[128n, 256f] -> [128f(part), 2 fh, 128n]."""
                ps = pool.tile([128, 512], f32, name="ps_tp", tag="ntp")
                for fh in range(2):
                    nc.tensor.transpose(ps[:, fh * 128:(fh + 1) * 128],
                                        src_ap[:, fh * 128:(fh + 1) * 128],
                                        ey32)
                t = cpool.tile([128, 2, 128], dt_out, name=out_name)
                nc.vector.tensor_copy(t[:].rearrange("p a b -> p (a b)"),
                                      ps[:, :256])
                return t

            # ---------- node stage 1 ----------
            with (
                tc.tile_pool(name="n1pre", bufs=2, space="PSUM") as npre,
                tc.tile_pool(name="n1tp", bufs=2, space="PSUM") as ntp,
            ):
                ps_x = ntp.tile([128, 512], f32, name="ps_x", tag="ntp")
                nc.tensor.transpose(ps_x[:, 0:128], x_sb[:, 0:128], ey32)
                nc.tensor.transpose(ps_x[0:68, 128:256], x_sb[:, 128:196],
                                    ey32)
                xt0 = cpool.tile([128, 128], f32, name="xt0")
                nc.vector.tensor_copy(xt0[:], ps_x[:, 0:128])
                xt1 = cpool.tile([68, 128], f32, name="xt1")
                nc.vector.tensor_copy(xt1[:], ps_x[0:68, 128:256])

                ps1 = npre.tile([128, 512], f32, name="ps_n", tag="npre")
                nc.tensor.matmul(ps1[:, :256], xt0[:], wn1a[:],
                                 start=True, stop=False)
                nc.tensor.matmul(ps1[:, :256], xt1[:], wn1b[:],
                                 start=False, stop=True)
                zh1a = elu_node(ps1, nbs[:, 0, :], "zh1a")
                zh1aT = tpose256(ntp, zh1a[:], "zh1aT")

                ps2 = node_mm(npre, zh1aT, wn1l2)
                zh1 = elu_node(ps2, nbs[:, 1, :], "zh1")
                zh1T = tpose256(ntp, zh1[:], "zh1T")

                ps_u1 = node_mm(npre, zh1T, a1s)
                u1sb = cpool.tile([128, 256], f32, name="u1sb")
                nc.scalar.copy(u1sb[:], ps_u1[:, :256])
                u1T = tpose256(ntp, u1sb[:], "u1T")
                ps_v1 = node_mm(npre, zh1T, b1s)
                v1sb = cpool.tile([128, 256], f32, name="v1sb")
                nc.scalar.copy(v1sb[:], ps_v1[:, :256])
                v1T = tpose256(ntp, v1sb[:], "v1T")

                # derived per-node tiles for pass 1 (tiny)
                e_u1 = cpool.tile([128, 2, 128], f32, name="e_u1")
                e_v1 = cpool.tile([128, 2, 128], f16, name="e_v1")
                u1p = cpool.tile([128, 2, 128], f32, name="u1p")
                v1t6 = cpool.tile([128, 2, 128], f16, name="v1t6")
                for fh in range(2):
                    nc.scalar.activation(e_u1[:, fh, :], u1T[:, fh, :],
                                         AF.Exp, bias=sc(0 + fh))
                    nc.scalar.activation(e_v1[:, fh, :], v1T[:, fh, :],
                                         AF.Exp)
                    nc.vector.tensor_scalar(u1p[:, fh, :], u1T[:, fh, :],
                                            sc(2 + fh), None, ALU.add)
                    nc.vector.tensor_copy(v1t6[:, fh, :], v1T[:, fh, :])

            # ---------- pass 1 over the padded pair grid ----------
            NB1 = C1 // 128  # 8 blocks per chunk
            with tc.tile_pool(name="p1ps", bufs=2, space="PSUM") as p1ps:
                def p1_stageA(c):
                    """t1 = e_u (x) e_v [DVE 4x]; r1 = max(u'+v+1,1) [Pool];
                    z1a = min(t1, r1) [DVE 2x]."""
                    t1 = wk.tile([128, 2, NB1, 128], f16, name="t1", tag="t1",
                                 bufs=2)
                    r1 = wk.tile([128, 2, NB1, 128], f16, name="r1", tag="r1",
                                 bufs=2)
                    for fh in range(2):
                        for b in range(NB1):
                            col = c * NB1 + b
                            nc.gpsimd.tensor_scalar(
                                r1[:, fh, b, :], v1t6[:, fh, :],
                                u1p[:, fh, col:col + 1], 1.0,
                                ALU.add, ALU.max)
                            nc.vector.tensor_scalar(
                                t1[:, fh, b, :], e_v1[:, fh, :],
                                e_u1[:, fh, col:col + 1], None, ALU.mult)
                    z1a = wk.tile([128, 2, C1], f16, name="z1a", tag="z1a",
                                  bufs=2)
                    nc.vector.tensor_tensor(
                        z1a[:].rearrange("p a b -> p (a b)"),
                        t1[:].rearrange("p a b j -> p (a b j)"),
                        r1[:].rearrange("p a b j -> p (a b j)"), ALU.min)
                    return z1a

                def p1_stageB(c, z1a):
                    ps = p1ps.tile([128, 2, C1], f32, name="ps1b", tag="p1")
                    for oh in range(2):
                        for q in range(0, C1, 512):
                            for fh in range(2):
                                nc.tensor.matmul(
                                    ps[:, oh, q:q + 512],
                                    we1l2[:, fh, oh * 128:(oh + 1) * 128],
                                    z1a[:, fh, q:q + 512],
                                    start=(fh == 0), stop=(fh == 1))
                    t1b = wk.tile([128, 2, C1], f16, name="t1b", tag="t1b",
                                  bufs=2)
                    r0b = wk.tile([128, 2, C1], f16, name="r0b", tag="r0b",
                                  bufs=2)
                    for oh in range(2):
                        nc.scalar.activation(t1b[:, oh, :], ps[:, oh, :],
                                             AF.Exp, bias=sc(4 + oh))
                        nc.scalar.activation(r0b[:, oh, :], ps[:, oh, :],
                                             AF.Relu, bias=sc(4 + oh))
                    rp = wk.tile([128, 2, C1], f16, name="rp", tag="rp",
                                 bufs=2)
                    nc.vector.tensor_scalar(
                        rp[:].rearrange("p a b -> p (a b)"),
                        r0b[:].rearrange("p a b -> p (a b)"),
                        1.0, None, ALU.add)
                    zs = z1[:, :, c * C1:(c + 1) * C1]
                    nc.vector.tensor_tensor(zs, t1b[:], rp[:], ALU.min)
                    # agg tree: per-block sums of z1 -> aggt columns
                    zb = zs.rearrange("p h (b j) -> p h b j", j=128)
                    s64 = wk.tile([128, 2, NB1, 64], f16, name="s64",
                                  tag="s64", bufs=2)
                    nc.vector.tensor_tensor(
                        s64[:], zb[:, :, :, 0:64], zb[:, :, :, 64:128],
                        ALU.add)
                    s32 = wk.tile([128, 2, NB1, 32], f16, name="s32",
                                  tag="s32", bufs=2)
                    nc.vector.tensor_tensor(
                        s32[:], s64[:, :, :, 0:32], s64[:, :, :, 32:64],
                        ALU.add)
                    s16 = wk.tile([128, 2, NB1, 16], f16, name="s16",
                                  tag="s16", bufs=2)
                    nc.vector.tensor_tensor(
                        s16[:], s32[:, :, :, 0:16], s32[:, :, :, 16:32],
                        ALU.add)
                    nc.vector.reduce_sum(
                        aggt[:, :, c * NB1:(c + 1) * NB1], s16[:],
                        axis=mybir.AxisListType.X)

                hist = []
                for c in range(P // C1):
                    hist.append(p1_stageA(c))
                    if c >= 1:
                        p1_stageB(c - 1, hist[c - 1])
                p1_stageB(P // C1 - 1, hist[-1])

            # diagonal correction: agg counted z1[i,i]; subtract it
            diag16 = cpool.tile([128, 2, 128], f16, name="diag16")
            zflat = z1[:].rearrange("p h e -> p (h e)")
            for h in range(2):
                nc.gpsimd.tensor_copy(
                    diag16[:, h, :],
                    zflat[:, h * P:h * P + P:129])
            aggc = cpool.tile([128, 2, 128], f32, name="aggc")
            nc.vector.tensor_tensor(
                aggc[:].rearrange("p a b -> p (a b)"),
                aggt[:].rearrange("p a b -> p (a b)"),
                diag16[:].rearrange("p a b -> p (a b)"), ALU.subtract)

            # ---------- node stage 2 ----------
            with (
                tc.tile_pool(name="n2pre", bufs=2, space="PSUM") as npre2,
                tc.tile_pool(name="n2tp", bufs=2, space="PSUM") as ntp2,
            ):
                ps3 = node_mm(npre2, aggc, wn2l1)
                zh2a = elu_node(ps3, nbs[:, 2, :], "zh2a")
                zh2aT = tpose256(ntp2, zh2a[:], "zh2aT")
                ps4 = node_mm(npre2, zh2aT, wn2l2)
                zh2 = elu_node(ps4, nbs[:, 3, :], "zh2")
                zh2T = tpose256(ntp2, zh2[:], "zh2T")

                ps_u2 = node_mm(npre2, zh2T, a2s)
                u2y = wk.tile([128, 256], f32, name="u2y", tag="n_y")
                nc.vector.tensor_tensor(u2y[:], ps_u2[:, :256], b3t, ALU.add)
                u2p6 = cpool.tile([128, 256], f16, name="u2p6")
                nc.scalar.copy(u2p6[:], u2y[:])
                ps_v2 = node_mm(npre2, zh2T, b2s)
                v2n6 = cpool.tile([128, 256], f16, name="v2n6")
                nc.scalar.copy(v2n6[:], ps_v2[:, :256])

            # ---------- pass 2 ----------
            NB2 = C2C // 128  # 4 blocks per chunk
            with (
                tc.tile_pool(name="p2a", bufs=2, space="PSUM") as p2a,
                tc.tile_pool(name="p2b", bufs=1, space="PSUM") as p2b,
                tc.tile_pool(name="p2o", bufs=2, space="PSUM") as p2o,
            ):
                def p2_A1_mm(c):
                    lo = c * C2C
                    psA = p2a.tile([128, 2, C2C], f32, name="psA", tag="p2a")
                    uey = wk.tile([128, C2C], f16, name="uey", tag="uey",
                                  bufs=4)
                    nc.sync.dma_start(uey[:], ueye_d[:, lo:lo + C2C])
                    for fh in range(2):
                        fsl = slice(fh * 128, (fh + 1) * 128)
                        for hh in range(2):
                            nc.tensor.matmul(
                                psA[:, fh, :], c2s[:, hh, fsl],
                                z1[:, hh, lo:lo + C2C],
                                start=(hh == 0), stop=False)
                        nc.tensor.matmul(
                            psA[:, fh, :], v2n6[:, fsl], eyet[:],
                            start=False, stop=False)
                        nc.tensor.matmul(
                            psA[:, fh, :], u2p6[:, fsl], uey[:],
                            start=False, stop=True)
                    return psA

                def p2_A2_elu(c, psA):
                    # psA = y2 + 1
                    psAf = psA[:].rearrange("p a b -> p (a b)")
                    t2 = wk.tile([128, 2, C2C], f16, name="t2", tag="t2",
                                 bufs=2)
                    t2f = t2[:].rearrange("p a b -> p (a b)")
                    nc.scalar.activation(t2f, psAf, AF.Exp, bias=sc(12))
                    t2p = wk.tile([128, 2, C2C], f16, name="t2p", tag="t2p",
                                  bufs=2)
                    t2pf = t2p[:].rearrange("p a b -> p (a b)")
                    nc.vector.tensor_scalar(t2pf, t2f, 1.0, None, ALU.min)
                    # z2a = max(min(exp(y2),1), y2+1)
                    z2a = wk.tile([128, 2, C2C], f16, name="z2a", tag="z2a",
                                  bufs=3)
                    nc.vector.tensor_tensor(
                        z2a[:].rearrange("p a b -> p (a b)"), psAf, t2pf,
                        ALU.max)
                    return z2a

                def p2_B1_mm(c, z2a):
                    psB = p2b.tile([128, 2, C2C], f32, name="psB", tag="p2b")
                    for oh in range(2):
                        for fh in range(2):
                            nc.tensor.matmul(
                                psB[:, oh, :],
                                we2l2[:, fh, oh * 128:(oh + 1) * 128],
                                z2a[:, fh, :],
                                start=(fh == 0), stop=(fh == 1))
                    return psB

                def p2_B2_elu(c, psB):
                    # p2B sum-form: z2 = exp(m) + relu(y2b') with
                    # m = min(y2b + be4, 0); relu(y2b') = (y2b + be4) - m.
                    # Linear term folds through the output matmul:
                    # y2b^T ow = z2a^T (e2w2@ow); be4^T ow -> bos.
                    q2 = wk.tile([128, 2, C2C], f16, name="q2", tag="q2",
                                 bufs=3)
                    neg = True
                    if not neg:
                        for oh in range(2):
                            nc.scalar.activation(q2[:, oh, :], psB[:, oh, :],
                                                 AF.Relu, bias=sc(13 + oh),
                                                 scale=-1.0)
                    else:
                        # q2 holds m = min(y2b + be4, 0)
                        for oh in range(2):
                            nc.vector.tensor_scalar(
                                q2[:, oh, :], psB[:, oh, :], sc(8 + oh), 0.0,
                                ALU.add, ALU.min)
                    t2b = wk.tile([128, 2, C2C], f16, name="t2b", tag="t2b",
                                  bufs=3)
                    nc.scalar.activation(
                        t2b[:].rearrange("p a b -> p (a b)"),
                        q2[:].rearrange("p a b -> p (a b)"), AF.Exp,
                        scale=(1.0 if neg else -1.0))
                    return (q2, t2b, neg)

                def p2_B3_out(c, z2a, qt):
                    q2, t2b, neg = qt
                    lo = c * C2C
                    op = p2o.tile([128, NB2 * NE], f32, name="op",
                                  tag="p2o")
                    for j in range(NB2):
                        jsl = slice(j * 128, (j + 1) * 128)
                        for hh in range(2):
                            nc.tensor.matmul(
                                op[:, 2 * j:2 * j + 2], t2b[:, hh, jsl],
                                ows[:, hh, :],
                                start=(hh == 0), stop=False)
                            nc.tensor.matmul(
                                op[:, 2 * j:2 * j + 2], q2[:, hh, jsl],
                                (owsn if neg else ows)[:, hh, :],
                                start=False, stop=False)
                            nc.tensor.matmul(
                                op[:, 2 * j:2 * j + 2], z2a[:, hh, jsl],
                                ws2[:, hh, :],
                                start=False, stop=(hh == 1))
                    osb = wk.tile([128, NB2 * NE], f32, name="osb",
                                  tag="osb", bufs=2)
                    nc.vector.tensor_tensor(osb[:], op[:], bos, ALU.add)
                    nc.sync.dma_start(
                        out_d[lo:lo + C2C, :].rearrange("(j p) c -> p j c",
                                                        p=128),
                        osb[:].rearrange("p (j c) -> p j c", c=NE))

                hist2 = []
                for c in range(P // C2C):
                    psA = p2_A1_mm(c)
                    hist2.append(p2_A2_elu(c, psA))
                    if c >= 1:
                        zz = hist2[c - 1]
                        p2_B3_out(c - 1, zz, p2_B2_elu(c - 1,
                                                      p2_B1_mm(c - 1, zz)))
                zz = hist2[-1]
                p2_B3_out(P // C2C - 1, zz,
                          p2_B2_elu(P // C2C - 1,
                                    p2_B1_mm(P // C2C - 1, zz)))

    nc.compile()
    return nc


def _prep_inputs(inputs):
    """Host-side constant preprocessing -> shared in_map (all cores)."""
    f = lambda a: np.ascontiguousarray(np.asarray(a, dtype=np.float32))
    cs = lambda w: w.sum(axis=0)

    n1w1, n1b1 = f(inputs["n1w1"]), f(inputs["n1b1"])
    n1w2, n1b2 = f(inputs["n1w2"]), f(inputs["n1b2"])
    e1w1, e1b1 = f(inputs["e1w1"]), f(inputs["e1b1"])
    e1w2, e1b2 = f(inputs["e1w2"]), f(inputs["e1b2"])
    n2w1, n2b1 = f(inputs["n2w1"]), f(inputs["n2b1"])
    n2w2, n2b2 = f(inputs["n2w2"]), f(inputs["n2b2"])
    e2w1, e2b1 = f(inputs["e2w1"]), f(inputs["e2b1"])
    e2w2, e2b2 = f(inputs["e2w2"]), f(inputs["e2b2"])
    ow, ob = f(inputs["ow"]), f(inputs["ob"])

    A1, B1 = e1w1[:256], e1w1[256:]
    A2, B2, C2m = e2w1[:256], e2w1[256:512], e2w1[512:]

    e1w2_h = e1w2.astype(np.float16)
    C2_h = C2m.astype(np.float16)
    e2w2_h = e2w2.astype(np.float16)
    ow_h = ow.astype(np.float16)

    # activations are stored as z = elu+1; fold the -1 into consumer biases
    be1 = e1b1 - cs(A1) - cs(B1)
    be2 = e1b2 - cs(e1w2_h.astype(np.float32))
    be3 = e2b1 - cs(A2) - cs(B2) - cs(C2_h.astype(np.float32))
    be4 = e2b2 - cs(e2w2_h.astype(np.float32))
    ob_adj = ob - cs(ow_h.astype(np.float32))

    indeg = 127.0
    nbias = np.zeros((128, 4, 256), np.float32)
    nbias[:, 0, :] = n1b1[None, :]
    nbias[:, 1, :] = (n1b2 - cs(n1w2))[None, :]
    nbias[:, 2, :] = (n2b1 - indeg * cs(n2w1))[None, :]
    nbias[:, 3, :] = (n2b2 - cs(n2w2))[None, :]

    def sqh(w):  # [256, x] -> [128, 2*x] partition-major halves
        return np.ascontiguousarray(
            w.reshape(2, 128, -1).transpose(1, 0, 2).reshape(128, -1))

    pk32 = np.zeros((128, CC32), np.float32)

    def put32(name, arr):
        c0, w = PK32[name]
        pk32[:arr.shape[0], c0:c0 + w] = arr

    put32("ey32", np.eye(128, dtype=np.float32))
    put32("wn1a", n1w1[:128])
    put32("wn1b", n1w1[128:])
    put32("wn1l2", sqh(n1w2))
    put32("a1s", sqh(A1)); put32("b1s", sqh(B1))
    put32("wn2l1", sqh(n2w1)); put32("wn2l2", sqh(n2w2))
    put32("a2s", sqh(A2)); put32("b2s", sqh(B2))
    put32("nbs", nbias.reshape(128, -1))
    put32("b3t", np.tile((e2b1 - cs(A2) - cs(B2) - cs(C2_h.astype(np.float32))
                          + 1.0)[None, :], (128, 1)))
    bos_v = ob_adj + be4 @ ow_h.astype(np.float32)
    put32("bos", np.tile(bos_v[None, :], (128, 4)).astype(np.float32))
    scalv = np.zeros((128, 16), np.float32)
    for fh in range(2):
        sl = slice(fh * 128, (fh + 1) * 128)
        scalv[:, 0 + fh] = be1[sl]
        scalv[:, 2 + fh] = be1[sl] + 1.0
        scalv[:, 4 + fh] = be2[sl]
        scalv[:, 6 + fh] = be2[sl] + 1.0
        scalv[:, 8 + fh] = be4[sl]
        scalv[:, 10 + fh] = be4[sl] + 1.0
    scalv[:, 12] = -1.0
    for fh in range(2):
        scalv[:, 13 + fh] = -be4[fh * 128:(fh + 1) * 128]
    put32("scal", scalv)

    pk16 = np.zeros((128, CC16), np.float16)

    def put16(name, arr):
        c0, w = PK16[name]
        pk16[:arr.shape[0], c0:c0 + w] = arr

    put16("we1l2", sqh(e1w2_h.astype(np.float32)).astype(np.float16))
    put16("c2s", sqh(C2_h.astype(np.float32)).astype(np.float16))
    put16("we2l2", sqh(e2w2_h.astype(np.float32)).astype(np.float16))
    put16("ows", sqh(ow_h.astype(np.float32)).astype(np.float16))
    put16("owsn", sqh(-ow_h.astype(np.float32)).astype(np.float16))
    ws2f = e2w2_h.astype(np.float32) @ ow_h.astype(np.float32)
    put16("ws2", sqh(ws2f).astype(np.float16))
    put16("eyet", np.tile(np.eye(128, dtype=np.float16), (1, 4)))

    ueye = np.repeat(np.eye(128, dtype=np.float16), 128, axis=1)

    shared = dict(pk32=pk32, pk16=pk16,
                  ueye=np.ascontiguousarray(ueye))
    return shared


def kernel(**inputs):
    global LAST_EXEC_NS
    if "prog" not in _PROG_CACHE:
        _PROG_CACHE["prog"] = _build_program()
    nc = _PROG_CACHE["prog"]

    shared = _prep_inputs(inputs)
    x = np.asarray(inputs["x"], dtype=np.float32)
    in_maps = []
    for b in range(B):
        m = dict(shared)
        m["x_nm"] = np.ascontiguousarray(x[b].reshape(N, F))
        in_maps.append(m)

    trace = os.environ.get("KERNEL_TRACE", "0") == "1"
    try:
        res = run_bass_kernel_spmd(nc, in_maps, core_ids=list(range(8)),
                                   trace=trace)
    except ModuleNotFoundError:
        res = run_bass_kernel_spmd(nc, in_maps, core_ids=list(range(8)),
                                   trace=False)
    if trace and res.exec_time_ns is not None:
        LAST_EXEC_NS = res.exec_time_ns
        print(f"HW exec time: {res.exec_time_ns} ns "
              f"(mean {res.mean_exec_time_ns} ns, "
              f"slowest core {res.max_exec_time_core_id})")

    keep = ~np.eye(N, dtype=bool)
    outs = []
    for b in range(B):
        padded = res.results[b]["out"].reshape(N, N, NE)
        outs.append(padded[keep])
    return np.stack(outs, axis=0).astype(np.float32)
